# revision 39
# baseline (speedup 1.0000x reference)
"""Masked dot-product attention (B=64, Lq=Lk=1024, d=64, fp32) on 8 TRN2 cores.

Strategy (batch-parallel, 8 batch slots per core):
  - Host prep folds the scale (1/sqrt(d)) into Q and the additive key mask
    into an extra contraction row, so masked scores come out of one matmul:
        S^T[k, q] = sum_{d<64} K[k,d] * Q[q,d]/8  +  maskadd[k] * 1
    with lhsT = ktm[:, ktile] ([65, 128], rows 0..63 = K^T, row 64 = maskadd)
    and rhs = qt ([65, 1024], rows 0..63 = Q^T/8, row 64 = ones).
  - exp on ScalarE (PSUM -> SBUF). No max-subtraction: logits are O(6)
    bounded and masked entries underflow exp() to exactly 0.  valid_len==0
    batches are host-patched (Q rows zeroed, mask zeroed) so scores are all 0
    -> uniform attention, matching the reference's softmax of constant -1e6.
  - O'[j, q] = sum_k vp[k, j] * P^T[k, q] accumulated over ktiles, where
    vp has a ones-column (j=64) producing the softmax denominators.
  - O' ([65, 1024] per batch: numerators + denominator row) is DMA'd out
    unnormalized; the divide + [j, q] -> [q, j] transpose happen on the host.
  - Raggedness: k-tiles fully beyond valid_len are dead (exp == 0), so each
    batch only needs nact = ceil(valid_len/128) k-tiles.  Batches are sorted
    by nact and dealt across cores so slot s runs max-of-group tiles on every
    core; the per-slot tile counts are baked into the program (compiled per
    distinct count tuple, cached).
Matmuls use float32r (1 cycle/row at N=512 vs 4 for fp32).
"""

import numpy as np

import concourse.bass as bass
import concourse.mybir as mybir
import concourse.tile as tile
from concourse import bacc
from concourse.bass_utils import run_bass_kernel_spmd

N_CORES = 8
B = 64
L = 1024
D = 64
BPC = B // N_CORES  # batch slots per core
KT = L // 128       # max k-tiles per batch
NEG_INF = -1000000.0

F32 = mybir.dt.float32
F32R = mybir.dt.float32r

_prog_cache = {}


def _build_program(ns):
    """ns: per-slot k-tile counts (tuple of BPC ints in 1..KT)."""
    nc = bacc.Bacc("TRN2", target_bir_lowering=False, debug=False,
                   num_devices=N_CORES)
    # qkt packs [ktm_ktile0 (128) | qt (1024) | ktm_ktile1.. (896)] so a
    # batch's whole Q/K working set arrives in one DMA (one HWDGE slot).
    qkt_d = nc.dram_tensor("qkt", [BPC, D + 1, 2 * L + 128], F32R,
                           kind="ExternalInput")
    vp_d = nc.dram_tensor("vp", [BPC, 128, KT, D + 1], F32R, kind="ExternalInput")
    o_d = nc.dram_tensor("o", [BPC, D + 1, L], F32, kind="ExternalOutput")

    with tile.TileContext(nc) as tc:
        with (
            tc.tile_pool(name="qk", bufs=8) as qk_pool,
            tc.tile_pool(name="vpp", bufs=8) as vp_pool,
            tc.tile_pool(name="pt", bufs=6) as pt_pool,
            tc.tile_pool(name="osb", bufs=4) as osb_pool,
            tc.tile_pool(name="sp", bufs=2, space="PSUM") as sp_pool,
            tc.tile_pool(name="op", bufs=2, space="PSUM") as op_pool,
        ):
            for b in range(BPC):
                nkt = ns[b]
                # Split loads, first-needed first, so the first matmul and
                # exp of the batch can start as early as possible.
                end = 128 + L + (nkt - 1) * 128
                qkt_s = qk_pool.tile([D + 1, 2 * L + 128], F32R, tag="qkt")
                vp_s = vp_pool.tile([128, KT, D + 1], F32R)
                if b == 0:
                    # head: ktm ktile0 + first half of qt -> first matmuls
                    nc.sync.dma_start(qkt_s[:, :640], qkt_d[b][:, :640])
                    nc.sync.dma_start(qkt_s[:, 640:end], qkt_d[b][:, 640:end])
                else:
                    nc.sync.dma_start(qkt_s[:, :end], qkt_d[b][:, :end])
                nc.sync.dma_start(vp_s[:, :1, :], vp_d[b][:, :1, :])
                if nkt > 1:
                    nc.sync.dma_start(vp_s[:, 1:nkt, :], vp_d[b][:, 1:nkt, :])
                qt_s = qkt_s[:, 128:128 + L]

                def ktm_sl(kt):
                    if kt == 0:
                        return qkt_s[:, :128]
                    o = 128 + L + (kt - 1) * 128
                    return qkt_s[:, o:o + 128]

                opsum = op_pool.tile([D + 1, L], F32)

                # Scores for this batch form a [128, nkt*1024] strip (k-tile
                # major, q within).  Pack it into [128, SEG]-column PSUM/SBUF
                # tiles so each exp instruction covers more
                # elements, amortizing the ~300ns ACT per-instruction ramp.
                # S-matmul jobs: (ktile, q-offset, width); the very first
                # tile of the program uses quarter-width matmuls + split exp
                # so the exp stream starts as soon as possible.
                SEG = 1024
                jobs = []
                for kt in range(nkt):
                    if b == 0 and kt == 0:
                        jobs += [(0, q * 256, 256) for q in range(4)]
                    else:
                        jobs += [(kt, h * 512, 512) for h in range(2)]
                total_cols = nkt * 1024
                pt_tiles = []
                gcol = 0
                sp = pt = None
                for (kt, qoff, w) in jobs:
                    off = gcol % SEG
                    if off == 0:
                        sp = sp_pool.tile([128, SEG], F32)
                        pt = pt_pool.tile([128, SEG], F32R)
                        pt_tiles.append(pt)
                    nc.tensor.matmul(
                        sp[:, off:off + w],
                        ktm_sl(kt),
                        qt_s[:, qoff:qoff + w],
                        start=True, stop=True,
                    )
                    gcol += w
                    fill = gcol % SEG or SEG
                    if fill == SEG or gcol == total_cols:
                        if b == 0 and len(pt_tiles) == 1:
                            for e0 in range(0, fill, 512):
                                e1 = min(e0 + 512, fill)
                                nc.scalar.activation(
                                    pt[:, e0:e1], sp[:, e0:e1],
                                    mybir.ActivationFunctionType.Exp)
                        else:
                            nc.scalar.activation(
                                pt[:, :fill], sp[:, :fill],
                                mybir.ActivationFunctionType.Exp)

                for kt in range(nkt):
                    vt = vp_s[:, kt, :]
                    for h in range(2):
                        g = kt * 1024 + h * 512
                        t, off = divmod(g, SEG)
                        nc.tensor.matmul(
                            opsum[:, h * 512:(h + 1) * 512],
                            vt,
                            pt_tiles[t][:, off:off + 512],
                            start=(kt == 0), stop=(kt == nkt - 1),
                        )

                if b == BPC - 1:
                    # final batch: one wide ACT copy + one DMA minimizes the
                    # post-stream tail (fewer sem hops / queue slots)
                    osb = osb_pool.tile([D + 1, L], F32, tag="osbw")
                    nc.scalar.copy(osb[:], opsum[:])
                    nc.sync.dma_start(o_d[b], osb[:])
                else:
                    for h in range(2):
                        sl = slice(h * 512, (h + 1) * 512)
                        osb = osb_pool.tile([D + 1, 512], F32, tag=f"osb{h}")
                        nc.vector.tensor_copy(osb[:], opsum[:, sl])
                        nc.sync.dma_start(o_d[b][:, sl], osb[:])

    nc.compile()
    return nc


def get_program(ns):
    ns = tuple(ns)
    if ns not in _prog_cache:
        _prog_cache[ns] = _build_program(ns)
    return _prog_cache[ns]


def _prep_inputs(q, k, v, vl):
    """q,k,v: [n, L, D] fp32; vl: [n] int. Returns (qkt, vp) arrays."""
    n = q.shape[0]
    qt = np.empty((n, D + 1, L), np.float32)
    qt[:, :D] = q.transpose(0, 2, 1) * np.float32(1.0 / np.sqrt(D))
    qt[:, D] = 1.0
    ktm = np.empty((n, D + 1, L), np.float32)
    ktm[:, :D] = k.transpose(0, 2, 1)
    iota = np.arange(L)
    ktm[:, D] = np.where(iota[None, :] < vl[:, None], 0.0, NEG_INF)
    # valid_len == 0: reference softmaxes a constant -1e6 row -> uniform.
    # Reproduce by zeroing the logits entirely (Q rows and mask row).
    zmask = vl == 0
    if zmask.any():
        qt[zmask, :D] = 0.0
        ktm[zmask, D] = 0.0
    qkt = np.empty((n, D + 1, 2 * L + 128), np.float32)
    qkt[:, :, :128] = ktm[:, :, :128]
    qkt[:, :, 128:128 + L] = qt
    qkt[:, :, 128 + L:2 * L] = ktm[:, :, 128:]
    qkt[:, :, 2 * L:] = 0.0
    vp = np.empty((n, L, D + 1), np.float32)
    vp[:, :, :D] = v
    vp[:, :, D] = 1.0
    vp = np.ascontiguousarray(
        vp.reshape(n, KT, 128, D + 1).transpose(0, 2, 1, 3))
    return qkt, vp


def kernel(queries, keys, values, valid_lens):
    queries = np.asarray(queries, np.float32)
    keys = np.asarray(keys, np.float32)
    values = np.asarray(values, np.float32)
    vl = np.asarray(valid_lens).astype(np.int64)

    # Ragged load balancing: sort batches by active k-tile count descending,
    # deal them across cores (slot s <- sorted[s*N_CORES + c]), so each slot
    # runs the max tile count of its group of 8 on every core.
    nact = np.where(vl == 0, KT, -(-vl // 128)).astype(np.int64)
    order = np.argsort(-nact, kind="stable")
    ns = tuple(int(nact[order[s * N_CORES]]) for s in range(BPC))

    qkt, vp = _prep_inputs(queries[order], keys[order], values[order],
                           vl[order])

    nc = get_program(ns)
    in_maps = []
    for c in range(N_CORES):
        idx = [s * N_CORES + c for s in range(BPC)]
        in_maps.append({
            "qkt": np.ascontiguousarray(qkt[idx]),
            "vp": np.ascontiguousarray(vp[idx]),
        })

    res = None
    for attempt in range(3):
        try:
            res = run_bass_kernel_spmd(nc, in_maps, list(range(N_CORES)))
            break
        except Exception:
            # Transient NRT/axon device failures have been observed on the
            # first execution of a freshly compiled NEFF; reset and retry.
            if attempt == 2:
                raise
            import time as _time
            _time.sleep(2.0)
            try:
                import jax
                jax.clear_caches()
            except Exception:
                pass

    out = np.empty((B, L, D), np.float32)
    for c in range(N_CORES):
        o = res.results[c]["o"]  # [BPC, D+1, L]: numerators + denom row
        on = (o[:, :D, :] / o[:, D:D + 1, :]).transpose(0, 2, 1)
        for s in range(BPC):
            out[order[s * N_CORES + c]] = on[s]
    return out
